# revision 8
# baseline (speedup 1.0000x reference)
"""Trainium2 Bass kernel: batched attention  out = softmax(Q K^T) V  (no 1/sqrt(d) scale).

Shapes (hardcoded): Q, K, V: [4, 16, 2048, 128] fp32 -> out [4, 16, 2048, 128] fp32.

Sharding: B*H = 64 heads, data-parallel across 8 NeuronCores (8 heads per core).

Per-head device algorithm (transpose-free matmul layout, 16-bit PE with hi/lo
split for the accuracy-critical S = Q K^T):
  Host pre-transposes Q, K to [D, N] per head and splits each into fp16
  hi + lo parts (q = q1 + q2 exactly to ~22 mantissa bits). V is sent fp16
  (values O(1): fp16 range fine, 2^-11 rounding).
  For each 128-wide key chunk c (dropped q2*k2 term ~2^-22):
      S_T[c]  = k1c.T @ q1 + k1c.T @ q2 + k2c.T @ q1   -> PSUM [128k, q] fp32
      E[c]    = exp(S_T[c])  (ACT; bf16 out -- bf16 covers exp range e^+-70;
                no max-subtract needed)
      O_T    += vc.T @ E[c]                     (PSUM accumulate, fp32)
      l4[g]  += ones.T @ E[c],  g = c mod 4     (4-way column-tiled row sums:
                the 4 M=1 matmuls stream concurrently in distinct PE column
                groups, output partitions 0/32/64/96)
  l = mask4.T @ l4 (fp32 matmul combining the 4 partial rows)
  r = approx-reciprocal(l) (DVE, ~2 ULP); broadcast across partitions
  (GPSIMD); O_sb = O_T * r (DVE) -> DMA out as O_T [D, N]; host transposes.
"""

import sys

sys.path.insert(0, "/opt/trn_rl_repo")

import numpy as np
import ml_dtypes

import concourse.bass as bass
import concourse.tile as tile
from concourse import bacc, mybir
from concourse.bass_utils import run_bass_kernel_spmd

B, H, N, D = 4, 16, 2048, 128
NCORES = 8
HPC = (B * H) // NCORES  # heads per core = 8
P = 128                  # partitions
NK = N // P              # key chunks per head = 16
QH = 2                   # q halves (1024 each) to fit PSUM
QHW = N // QH            # 1024
F32 = mybir.dt.float32
BF16 = mybir.dt.bfloat16
FP16 = mybir.dt.float16


def build_nc():
    nc = bacc.Bacc(None, target_bir_lowering=False)

    q1_d = nc.dram_tensor("q1", [HPC, D, N], FP16, kind="ExternalInput")
    q2_d = nc.dram_tensor("q2", [HPC, D, N], FP16, kind="ExternalInput")
    k1_d = nc.dram_tensor("k1", [HPC, D, N], FP16, kind="ExternalInput")
    k2_d = nc.dram_tensor("k2", [HPC, D, N], FP16, kind="ExternalInput")
    v_d = nc.dram_tensor("v", [HPC, N, D], FP16, kind="ExternalInput")
    ot_d = nc.dram_tensor("ot", [HPC, D, N], F32, kind="ExternalOutput")

    with tile.TileContext(nc) as tc:
        with (
            tc.tile_pool(name="const", bufs=1) as const_pool,
            tc.tile_pool(name="io", bufs=2) as io_pool,
            tc.tile_pool(name="e", bufs=6) as e_pool,
            tc.tile_pool(name="osb", bufs=2) as o_pool,
            tc.tile_pool(name="small", bufs=2) as small_pool,
            tc.tile_pool(name="ps_s", bufs=2, space="PSUM") as ps_s_pool,
            tc.tile_pool(name="ps_o", bufs=1, space="PSUM") as ps_o_pool,
            tc.tile_pool(name="ps_l", bufs=1, space="PSUM") as ps_l_pool,
        ):
            ones_col = const_pool.tile([P, 1], FP16)  # sum weights
            nc.vector.memset(ones_col[:], 1.0)
            mask4 = const_pool.tile([P, 1], F32)      # combine weights
            nc.vector.memset(mask4[:], 0.0)
            for g in range(4):
                nc.vector.memset(mask4[32 * g: 32 * g + 1, :], 1.0)

            for h in range(HPC):
                q1t = io_pool.tile([P, N], FP16, tag="q1")
                nc.sync.dma_start(out=q1t[:], in_=q1_d[h])
                q2t = io_pool.tile([P, N], FP16, tag="q2")
                nc.sync.dma_start(out=q2t[:], in_=q2_d[h])
                k1t = io_pool.tile([P, N], FP16, tag="k1")
                nc.sync.dma_start(out=k1t[:], in_=k1_d[h])
                k2t = io_pool.tile([P, N], FP16, tag="k2")
                nc.sync.dma_start(out=k2t[:], in_=k2_d[h])
                # vt[p, c, d] = V[h, c*128 + p, d]
                vt3 = io_pool.tile([P, NK, P], FP16, tag="vt")
                nc.sync.dma_start(
                    out=vt3[:], in_=v_d[h].rearrange("(c p) d -> p c d", p=P)
                )
                vt = vt3.rearrange("p c d -> p (c d)")

                for qh in range(QH):
                    q0 = qh * QHW
                    ps_o = ps_o_pool.tile([P, QHW], F32, tag="o")
                    ps_l = ps_l_pool.tile([P, QHW], F32, tag="l")
                    for c in range(NK):
                        cs = slice(c * P, (c + 1) * P)
                        ps_s = ps_s_pool.tile([P, QHW], F32, tag="s")
                        # 3-term hi/lo split of S; grouped by lhsT for reuse
                        terms = [
                            (k1t[:, cs], q1t, False),
                            (k1t[:, cs], q2t, False),
                            (k2t[:, cs], q1t, True),
                        ]
                        for t, (kc, qt, last) in enumerate(terms):
                            for j in range(2):
                                sl = slice(j * 512, (j + 1) * 512)
                                nc.tensor.matmul(
                                    ps_s[:, sl],
                                    kc,
                                    qt[:, q0 + j * 512: q0 + (j + 1) * 512],
                                    start=(t == 0),
                                    stop=last,
                                )
                        e = e_pool.tile([P, QHW], BF16, tag="e")
                        nc.scalar.activation(
                            e[:], ps_s[:], mybir.ActivationFunctionType.Exp
                        )
                        for j in range(2):
                            sl = slice(j * 512, (j + 1) * 512)
                            nc.tensor.matmul(
                                ps_o[:, sl],
                                vt[:, cs],
                                e[:, sl],
                                start=(c == 0),
                                stop=(c == NK - 1),
                            )
                        # 4-way column-tiled row-sum: chunk c -> group c%4,
                        # output partition 32*(c%4). Groups of 4 consecutive
                        # chunks stream concurrently on the PE.
                        g = c % 4
                        for j in range(2):
                            sl = slice(j * 512, (j + 1) * 512)
                            nc.tensor.matmul(
                                ps_l[32 * g: 32 * g + 1, sl],
                                ones_col[:],
                                e[:, sl],
                                start=(c < 4),
                                stop=(c >= NK - 4),
                                tile_position=(0, 32 * g),
                            )
                    # combine 4 partial rows: l = mask4.T @ l4  (fp32 matmul)
                    l4_sb = small_pool.tile([P, QHW], F32, tag="l4")
                    nc.vector.tensor_copy(l4_sb[:], ps_l[:])
                    ps_lc = ps_s_pool.tile([P, QHW], F32, tag="s")
                    for j in range(2):
                        sl = slice(j * 512, (j + 1) * 512)
                        nc.tensor.matmul(
                            ps_lc[0:1, sl], mask4[:], l4_sb[:, sl],
                            start=True, stop=True,
                        )
                    # r = 1/l (DVE approx, ~2 ULP), broadcast across partitions
                    # (GPSIMD), then O = O_T * r (DVE) and store.
                    r_sb = small_pool.tile([1, QHW], F32, tag="r")
                    scratch = small_pool.tile([1, QHW], F32, tag="rs")
                    nc.vector.reciprocal_approx_accurate(
                        r_sb[:], ps_lc[0:1, :], scratch[:]
                    )
                    r_bc = small_pool.tile([P, QHW], F32, tag="rbc")
                    nc.gpsimd.partition_broadcast(r_bc[:], r_sb[:])
                    o_sb = o_pool.tile([P, QHW], F32, tag="osb")
                    nc.vector.tensor_mul(o_sb[:], ps_o[:], r_bc[:])
                    nc.sync.dma_start(out=ot_d[h][:, q0: q0 + QHW], in_=o_sb[:])
    nc.finalize()
    return nc


def _split_fp16_t(x):
    """[heads, N, D] fp32 -> transposed [heads, D, N] fp16 hi and lo parts."""
    xt = np.ascontiguousarray(x.transpose(0, 2, 1))
    hi = xt.astype(np.float16)
    lo = (xt - hi.astype(np.float32)).astype(np.float16)
    return hi, lo


def _prepare_in_maps(Q, K, V):
    Qf = np.asarray(Q, dtype=np.float32).reshape(B * H, N, D)
    Kf = np.asarray(K, dtype=np.float32).reshape(B * H, N, D)
    Vf = np.asarray(V, dtype=np.float32).reshape(B * H, N, D).astype(np.float16)
    q1, q2 = _split_fp16_t(Qf)
    k1, k2 = _split_fp16_t(Kf)
    in_maps = []
    for i in range(NCORES):
        s = slice(i * HPC, (i + 1) * HPC)
        in_maps.append(
            {"q1": q1[s], "q2": q2[s], "k1": k1[s], "k2": k2[s], "v": Vf[s]}
        )
    return in_maps


def run(Q, K, V, trace=False, **kwargs):
    nc = build_nc()
    in_maps = _prepare_in_maps(Q, K, V)
    res = run_bass_kernel_spmd(nc, in_maps, list(range(NCORES)), trace=trace, **kwargs)
    OT = np.concatenate([res.results[i]["ot"] for i in range(NCORES)], axis=0)
    out = OT.transpose(0, 2, 1).reshape(B, H, N, D)
    return np.ascontiguousarray(out), res


def kernel(Q, K, V):
    out, _ = run(Q, K, V, trace=False)
    return out


# revision 10
# speedup vs baseline: 1.0903x; 1.0903x over previous
"""Trainium2 Bass kernel: batched attention  out = softmax(Q K^T) V  (no 1/sqrt(d) scale).

Shapes (hardcoded): Q, K, V: [4, 16, 2048, 128] fp32 -> out [4, 16, 2048, 128] fp32.

Sharding: B*H = 64 heads, data-parallel across 8 NeuronCores (8 heads per core).

Per-head device algorithm (transpose-free matmul layout, 16-bit PE with hi/lo
split for the accuracy-critical S = Q K^T):
  Host pre-transposes Q, K to [D, N] per head and splits each into fp16
  hi + lo parts (q = q1 + q2 exactly to ~22 mantissa bits). V is sent fp16
  (values O(1): fp16 range fine, 2^-11 rounding).
  For each 128-wide key chunk c (dropped q2*k2 term ~2^-22):
      S_T[c]  = k1c.T @ q1 + k1c.T @ q2 + k2c.T @ q1   -> PSUM [128k, q] fp32
      E[c]    = exp(S_T[c])  (ACT; bf16 out -- bf16 covers exp range e^+-70;
                no max-subtract needed)
      O_T    += vc.T @ E[c]                     (PSUM accumulate, fp32)
      l4[g]  += ones.T @ E[c],  g = c mod 4     (4-way column-tiled row sums:
                the 4 M=1 matmuls stream concurrently in distinct PE column
                groups, output partitions 0/32/64/96)
  l = mask4.T @ l4 (fp32 matmul combining the 4 partial rows)
  r = approx-reciprocal(l) (DVE, ~2 ULP); broadcast across partitions
  (GPSIMD); O_sb = O_T * r (DVE) -> DMA out as O_T [D, N]; host transposes.
"""

import sys

sys.path.insert(0, "/opt/trn_rl_repo")

import numpy as np
import ml_dtypes

import concourse.bass as bass
import concourse.tile as tile
from concourse import bacc, mybir
from concourse.bass_utils import run_bass_kernel_spmd

B, H, N, D = 4, 16, 2048, 128
NCORES = 8
HPC = (B * H) // NCORES  # heads per core = 8
P = 128                  # partitions
NK = N // P              # key chunks per head = 16
QH = 2                   # q halves (1024 each) to fit PSUM
QHW = N // QH            # 1024
F32 = mybir.dt.float32
BF16 = mybir.dt.bfloat16
FP16 = mybir.dt.float16


def build_nc():
    nc = bacc.Bacc(None, target_bir_lowering=False)

    q1_d = nc.dram_tensor("q1", [HPC, D, N], FP16, kind="ExternalInput")
    q2_d = nc.dram_tensor("q2", [HPC, D, N], FP16, kind="ExternalInput")
    k1_d = nc.dram_tensor("k1", [HPC, D, N], FP16, kind="ExternalInput")
    k2_d = nc.dram_tensor("k2", [HPC, D, N], FP16, kind="ExternalInput")
    v_d = nc.dram_tensor("v", [HPC, N, D], FP16, kind="ExternalInput")
    ot_d = nc.dram_tensor("ot", [HPC, D, N], F32, kind="ExternalOutput")

    with tile.TileContext(nc) as tc:
        with (
            tc.tile_pool(name="const", bufs=1) as const_pool,
            tc.tile_pool(name="io", bufs=2) as io_pool,
            tc.tile_pool(name="e", bufs=6) as e_pool,
            tc.tile_pool(name="osb", bufs=2) as o_pool,
            tc.tile_pool(name="small", bufs=2) as small_pool,
            tc.tile_pool(name="ps_s", bufs=2, space="PSUM") as ps_s_pool,
            tc.tile_pool(name="ps_o", bufs=1, space="PSUM") as ps_o_pool,
            tc.tile_pool(name="ps_l", bufs=1, space="PSUM") as ps_l_pool,
        ):
            ones_col = const_pool.tile([P, 1], FP16)  # sum weights
            nc.vector.memset(ones_col[:], 1.0)
            mask4 = const_pool.tile([P, 1], F32)      # combine weights
            nc.vector.memset(mask4[:], 0.0)
            for g in range(4):
                nc.vector.memset(mask4[32 * g: 32 * g + 1, :], 1.0)

            for h in range(HPC):
                q1t = io_pool.tile([P, N], FP16, tag="q1")
                nc.sync.dma_start(out=q1t[:], in_=q1_d[h])
                q2t = io_pool.tile([P, N], FP16, tag="q2")
                nc.sync.dma_start(out=q2t[:], in_=q2_d[h])
                k1t = io_pool.tile([P, N], FP16, tag="k1")
                nc.sync.dma_start(out=k1t[:], in_=k1_d[h])
                k2t = io_pool.tile([P, N], FP16, tag="k2")
                nc.sync.dma_start(out=k2t[:], in_=k2_d[h])
                # vt[p, c, d] = V[h, c*128 + p, d]
                vt3 = io_pool.tile([P, NK, P], FP16, tag="vt")
                nc.sync.dma_start(
                    out=vt3[:], in_=v_d[h].rearrange("(c p) d -> p c d", p=P)
                )
                vt = vt3.rearrange("p c d -> p (c d)")

                for qh in range(QH):
                    q0 = qh * QHW
                    ps_o = ps_o_pool.tile([P, QHW], F32, tag="o")
                    ps_l = ps_l_pool.tile([P, QHW], F32, tag="l")
                    e_tiles = []
                    for c in range(NK):
                        cs = slice(c * P, (c + 1) * P)
                        ps_s = ps_s_pool.tile([P, QHW], F32, tag="s")
                        # 3-term hi/lo split of S; grouped by lhsT for reuse
                        terms = [
                            (k1t[:, cs], q1t, False),
                            (k1t[:, cs], q2t, False),
                            (k2t[:, cs], q1t, True),
                        ]
                        for t, (kc, qt, last) in enumerate(terms):
                            for j in range(2):
                                sl = slice(j * 512, (j + 1) * 512)
                                nc.tensor.matmul(
                                    ps_s[:, sl],
                                    kc,
                                    qt[:, q0 + j * 512: q0 + (j + 1) * 512],
                                    start=(t == 0),
                                    stop=last,
                                )
                        e = e_pool.tile([P, QHW], BF16, tag="e")
                        nc.scalar.activation(
                            e[:], ps_s[:], mybir.ActivationFunctionType.Exp
                        )
                        e_tiles.append(e)
                        for j in range(2):
                            sl = slice(j * 512, (j + 1) * 512)
                            nc.tensor.matmul(
                                ps_o[:, sl],
                                vt[:, cs],
                                e[:, sl],
                                start=(c == 0),
                                stop=(c == NK - 1),
                            )
                        # 4-way column-tiled row sums, batched so the four M=1
                        # matmuls (distinct PE column groups, output partitions
                        # 0/32/64/96) are issued back-to-back and stream
                        # concurrently through the array.
                        if c % 4 == 3:
                            for j in range(2):
                                sl = slice(j * 512, (j + 1) * 512)
                                for g in range(4):
                                    nc.tensor.matmul(
                                        ps_l[32 * g: 32 * g + 1, sl],
                                        ones_col[:],
                                        e_tiles[g][:, sl],
                                        start=(c == 3),
                                        stop=(c == NK - 1),
                                        tile_position=(0, 32 * g),
                                    )
                            e_tiles = []
                    # combine 4 partial rows: l = mask4.T @ l4  (fp32 matmul)
                    l4_sb = small_pool.tile([P, QHW], F32, tag="l4")
                    nc.vector.tensor_copy(l4_sb[:], ps_l[:])
                    ps_lc = ps_s_pool.tile([P, QHW], F32, tag="s")
                    for j in range(2):
                        sl = slice(j * 512, (j + 1) * 512)
                        nc.tensor.matmul(
                            ps_lc[0:1, sl], mask4[:], l4_sb[:, sl],
                            start=True, stop=True,
                        )
                    # r = 1/l (DVE approx, ~2 ULP), broadcast across partitions
                    # (GPSIMD), then O = O_T * r (DVE) and store.
                    r_sb = small_pool.tile([1, QHW], F32, tag="r")
                    scratch = small_pool.tile([1, QHW], F32, tag="rs")
                    nc.vector.reciprocal_approx_accurate(
                        r_sb[:], ps_lc[0:1, :], scratch[:]
                    )
                    r_bc = small_pool.tile([P, QHW], F32, tag="rbc")
                    nc.gpsimd.partition_broadcast(r_bc[:], r_sb[:])
                    o_sb = o_pool.tile([P, QHW], F32, tag="osb")
                    nc.vector.tensor_mul(o_sb[:], ps_o[:], r_bc[:])
                    nc.sync.dma_start(out=ot_d[h][:, q0: q0 + QHW], in_=o_sb[:])
    nc.finalize()
    return nc


def _split_fp16_t(x):
    """[heads, N, D] fp32 -> transposed [heads, D, N] fp16 hi and lo parts."""
    xt = np.ascontiguousarray(x.transpose(0, 2, 1))
    hi = xt.astype(np.float16)
    lo = (xt - hi.astype(np.float32)).astype(np.float16)
    return hi, lo


def _prepare_in_maps(Q, K, V):
    Qf = np.asarray(Q, dtype=np.float32).reshape(B * H, N, D)
    Kf = np.asarray(K, dtype=np.float32).reshape(B * H, N, D)
    Vf = np.asarray(V, dtype=np.float32).reshape(B * H, N, D).astype(np.float16)
    q1, q2 = _split_fp16_t(Qf)
    k1, k2 = _split_fp16_t(Kf)
    in_maps = []
    for i in range(NCORES):
        s = slice(i * HPC, (i + 1) * HPC)
        in_maps.append(
            {"q1": q1[s], "q2": q2[s], "k1": k1[s], "k2": k2[s], "v": Vf[s]}
        )
    return in_maps


def run(Q, K, V, trace=False, **kwargs):
    nc = build_nc()
    in_maps = _prepare_in_maps(Q, K, V)
    res = run_bass_kernel_spmd(nc, in_maps, list(range(NCORES)), trace=trace, **kwargs)
    OT = np.concatenate([res.results[i]["ot"] for i in range(NCORES)], axis=0)
    out = OT.transpose(0, 2, 1).reshape(B, H, N, D)
    return np.ascontiguousarray(out), res


def kernel(Q, K, V):
    out, _ = run(Q, K, V, trace=False)
    return out


# revision 13
# speedup vs baseline: 1.2015x; 1.1020x over previous
"""Trainium2 Bass kernel: batched attention  out = softmax(Q K^T) V  (no 1/sqrt(d) scale).

Shapes (hardcoded): Q, K, V: [4, 16, 2048, 128] fp32 -> out [4, 16, 2048, 128] fp32.

Sharding: B*H = 64 heads, data-parallel across 8 NeuronCores (8 heads per core).

Per-head device algorithm (transpose-free matmul layout, 16-bit PE with hi/lo
split for the accuracy-critical S = Q K^T):
  Host pre-transposes Q, K to [D, N] per head and splits each into fp16
  hi + lo parts (q = q1 + q2 exactly to ~22 mantissa bits). V is sent fp16
  (values O(1): fp16 range fine, 2^-11 rounding).
  For each 128-wide key chunk c (dropped q2*k2 term ~2^-22):
      S_T[c]  = k1c.T @ q1 + k1c.T @ q2 + k2c.T @ q1   -> PSUM [128k, q] fp32
      E[c]    = exp(S_T[c])  (ACT; bf16 out -- bf16 covers exp range e^+-70;
                no max-subtract needed)
      O_T    += vc.T @ E[c]                     (PSUM accumulate, fp32)
      l4[g]  += ones.T @ E[c],  g = c mod 4     (4-way column-tiled row sums:
                the 4 M=1 matmuls stream concurrently in distinct PE column
                groups, output partitions 0/32/64/96)
  l = mask4.T @ l4 (fp32 matmul combining the 4 partial rows)
  r = approx-reciprocal(l) (DVE, ~2 ULP); broadcast across partitions
  (GPSIMD); O_sb = O_T * r (DVE) -> DMA out as O_T [D, N]; host transposes.
"""

import sys

sys.path.insert(0, "/opt/trn_rl_repo")

import numpy as np
import ml_dtypes

import concourse.bass as bass
import concourse.tile as tile
from concourse import bacc, mybir
from concourse.bass_utils import run_bass_kernel_spmd

B, H, N, D = 4, 16, 2048, 128
NCORES = 8
HPC = (B * H) // NCORES  # heads per core = 8
P = 128                  # partitions
NK = N // P              # key chunks per head = 16
QH = 2                   # q halves (1024 each) to fit PSUM
QHW = N // QH            # 1024
F32 = mybir.dt.float32
BF16 = mybir.dt.bfloat16
FP16 = mybir.dt.float16


def build_nc():
    nc = bacc.Bacc(None, target_bir_lowering=False)

    q1_d = nc.dram_tensor("q1", [HPC, D, N], FP16, kind="ExternalInput")
    q2_d = nc.dram_tensor("q2", [HPC, D, N], FP16, kind="ExternalInput")
    k1_d = nc.dram_tensor("k1", [HPC, D, N], FP16, kind="ExternalInput")
    k2_d = nc.dram_tensor("k2", [HPC, D, N], FP16, kind="ExternalInput")
    v_d = nc.dram_tensor("v", [HPC, N, D], FP16, kind="ExternalInput")
    ot_d = nc.dram_tensor("ot", [HPC, D, N], F32, kind="ExternalOutput")

    with tile.TileContext(nc) as tc:
        with (
            tc.tile_pool(name="const", bufs=1) as const_pool,
            tc.tile_pool(name="io", bufs=2) as io_pool,
            tc.tile_pool(name="e", bufs=10) as e_pool,
            tc.tile_pool(name="osb", bufs=2) as o_pool,
            tc.tile_pool(name="small", bufs=2) as small_pool,
            tc.tile_pool(name="ps_s", bufs=2, space="PSUM") as ps_s_pool,
            tc.tile_pool(name="ps_o", bufs=1, space="PSUM") as ps_o_pool,
            tc.tile_pool(name="ps_l", bufs=1, space="PSUM") as ps_l_pool,
        ):
            ones_col = const_pool.tile([P, 1], FP16)  # sum weights
            nc.vector.memset(ones_col[:], 1.0)
            mask4 = const_pool.tile([P, 1], F32)      # combine weights
            nc.vector.memset(mask4[:], 0.0)
            for g in range(4):
                nc.vector.memset(mask4[32 * g: 32 * g + 1, :], 1.0)

            SUMB = 8  # chunks per column-tiled row-sum batch

            def load_head(h):
                q1t = io_pool.tile([P, N], FP16, tag="q1")
                nc.sync.dma_start(out=q1t[:], in_=q1_d[h])
                q2t = io_pool.tile([P, N], FP16, tag="q2")
                nc.sync.dma_start(out=q2t[:], in_=q2_d[h])
                k1t = io_pool.tile([P, N], FP16, tag="k1")
                nc.sync.dma_start(out=k1t[:], in_=k1_d[h])
                k2t = io_pool.tile([P, N], FP16, tag="k2")
                nc.sync.dma_start(out=k2t[:], in_=k2_d[h])
                # vt[p, c, d] = V[h, c*128 + p, d]
                vt3 = io_pool.tile([P, NK, P], FP16, tag="vt")
                nc.sync.dma_start(
                    out=vt3[:], in_=v_d[h].rearrange("(c p) d -> p c d", p=P)
                )
                return q1t, q2t, k1t, k2t, vt3.rearrange("p c d -> p (c d)")

            def make_tail(ps_o, ps_l, h, q0):
                def tail():
                    # combine 4 partial rows: l = mask4.T @ l4 (fp32 matmul),
                    # r = 1/l (DVE approx, ~2 ULP), broadcast across
                    # partitions (GPSIMD), O = O_T * r (DVE), store.
                    l4_sb = small_pool.tile([P, QHW], F32, tag="l4")
                    nc.vector.tensor_copy(l4_sb[:], ps_l[:])
                    ps_lc = ps_s_pool.tile([P, QHW], F32, tag="s")
                    for j in range(2):
                        sl = slice(j * 512, (j + 1) * 512)
                        nc.tensor.matmul(
                            ps_lc[0:1, sl], mask4[:], l4_sb[:, sl],
                            start=True, stop=True,
                        )
                    r_sb = small_pool.tile([1, QHW], F32, tag="r")
                    scratch = small_pool.tile([1, QHW], F32, tag="rs")
                    nc.vector.reciprocal_approx_accurate(
                        r_sb[:], ps_lc[0:1, :], scratch[:]
                    )
                    r_bc = small_pool.tile([P, QHW], F32, tag="rbc")
                    nc.gpsimd.partition_broadcast(r_bc[:], r_sb[:])
                    o_sb = o_pool.tile([P, QHW], F32, tag="osb")
                    nc.vector.tensor_mul(o_sb[:], ps_o[:], r_bc[:])
                    nc.sync.dma_start(out=ot_d[h][:, q0: q0 + QHW], in_=o_sb[:])
                return tail

            pending_tail = None
            tiles = None
            for h in range(HPC):
                for qh in range(QH):
                    if qh == 0:
                        tiles = load_head(h)
                    q1t, q2t, k1t, k2t, vt = tiles
                    q0 = qh * QHW
                    ps_o = ps_o_pool.tile([P, QHW], F32, tag="o")
                    ps_l = ps_l_pool.tile([P, QHW], F32, tag="l")
                    e_tiles = []
                    for c in range(NK):
                        cs = slice(c * P, (c + 1) * P)
                        ps_s = ps_s_pool.tile([P, QHW], F32, tag="s")
                        # 3-term hi/lo split of S; grouped by lhsT for reuse
                        terms = [
                            (k1t[:, cs], q1t, False),
                            (k1t[:, cs], q2t, False),
                            (k2t[:, cs], q1t, True),
                        ]
                        for t, (kc, qt, last) in enumerate(terms):
                            for j in range(2):
                                sl = slice(j * 512, (j + 1) * 512)
                                nc.tensor.matmul(
                                    ps_s[:, sl],
                                    kc,
                                    qt[:, q0 + j * 512: q0 + (j + 1) * 512],
                                    start=(t == 0),
                                    stop=last,
                                )
                        e = e_pool.tile([P, QHW], BF16, tag="e")
                        nc.scalar.activation(
                            e[:], ps_s[:], mybir.ActivationFunctionType.Exp
                        )
                        e_tiles.append(e)
                        for j in range(2):
                            sl = slice(j * 512, (j + 1) * 512)
                            nc.tensor.matmul(
                                ps_o[:, sl],
                                vt[:, cs],
                                e[:, sl],
                                start=(c == 0),
                                stop=(c == NK - 1),
                            )
                        # previous round's normalization tail, deferred here so
                        # its DVE/GPSIMD latency hides behind this round's
                        # S-matmul stream instead of stalling the PE.
                        if c == 1 and pending_tail is not None:
                            pending_tail()
                            pending_tail = None
                        # Column-tiled row sums, batched: the four M=1 matmul
                        # groups (PE column groups / output partitions
                        # 0/32/64/96) are issued in waves of 4 so distinct
                        # groups stream concurrently through the array.
                        if c % SUMB == SUMB - 1:
                            for j in range(2):
                                sl = slice(j * 512, (j + 1) * 512)
                                for rep in range(SUMB // 4):
                                    for g in range(4):
                                        nc.tensor.matmul(
                                            ps_l[32 * g: 32 * g + 1, sl],
                                            ones_col[:],
                                            e_tiles[rep * 4 + g][:, sl],
                                            start=(c == SUMB - 1 and rep == 0),
                                            stop=(
                                                c == NK - 1
                                                and rep == SUMB // 4 - 1
                                            ),
                                            tile_position=(0, 32 * g),
                                        )
                            e_tiles = []
                    pending_tail = make_tail(ps_o, ps_l, h, q0)
            pending_tail()
    nc.finalize()
    return nc


def _split_fp16_t(x):
    """[heads, N, D] fp32 -> transposed [heads, D, N] fp16 hi and lo parts."""
    xt = np.ascontiguousarray(x.transpose(0, 2, 1))
    hi = xt.astype(np.float16)
    lo = (xt - hi.astype(np.float32)).astype(np.float16)
    return hi, lo


def _prepare_in_maps(Q, K, V):
    Qf = np.asarray(Q, dtype=np.float32).reshape(B * H, N, D)
    Kf = np.asarray(K, dtype=np.float32).reshape(B * H, N, D)
    Vf = np.asarray(V, dtype=np.float32).reshape(B * H, N, D).astype(np.float16)
    q1, q2 = _split_fp16_t(Qf)
    k1, k2 = _split_fp16_t(Kf)
    in_maps = []
    for i in range(NCORES):
        s = slice(i * HPC, (i + 1) * HPC)
        in_maps.append(
            {"q1": q1[s], "q2": q2[s], "k1": k1[s], "k2": k2[s], "v": Vf[s]}
        )
    return in_maps


def run(Q, K, V, trace=False, **kwargs):
    nc = build_nc()
    in_maps = _prepare_in_maps(Q, K, V)
    res = run_bass_kernel_spmd(nc, in_maps, list(range(NCORES)), trace=trace, **kwargs)
    OT = np.concatenate([res.results[i]["ot"] for i in range(NCORES)], axis=0)
    out = OT.transpose(0, 2, 1).reshape(B, H, N, D)
    return np.ascontiguousarray(out), res


def kernel(Q, K, V):
    out, _ = run(Q, K, V, trace=False)
    return out


# revision 18
# speedup vs baseline: 1.2804x; 1.0656x over previous
"""Trainium2 Bass kernel: batched attention  out = softmax(Q K^T) V  (no 1/sqrt(d) scale).

Shapes (hardcoded): Q, K, V: [4, 16, 2048, 128] fp32 -> out [4, 16, 2048, 128] fp32.

Sharding: B*H = 64 heads, data-parallel across 8 NeuronCores (8 heads per core).

Per-head device algorithm (transpose-free matmul layout, 16-bit PE with hi/lo
split for the accuracy-critical S = Q K^T):
  Host pre-transposes Q, K to [D, N] per head and splits each into fp16
  hi + lo parts (q = q1 + q2 exactly to ~22 mantissa bits). V is sent fp16
  (values O(1): fp16 range fine, 2^-11 rounding).
  For each 128-wide key chunk c (dropped q2*k2 term ~2^-22):
      S_T[c]  = k1c.T @ q1 + k1c.T @ q2 + k2c.T @ q1   -> PSUM [128k, q] fp32
      E[c]    = exp(S_T[c])  (ACT; bf16 out -- bf16 covers exp range e^+-70;
                no max-subtract needed)
      O_T    += vc.T @ E[c]                     (PSUM accumulate, fp32)
      l4[g]  += ones.T @ E[c],  g = c mod 4     (4-way column-tiled row sums:
                the 4 M=1 matmuls stream concurrently in distinct PE column
                groups, output partitions 0/32/64/96)
  l = mask4.T @ l4 (fp32 matmul combining the 4 partial rows)
  r = approx-reciprocal(l) (DVE, ~2 ULP); broadcast across partitions
  (GPSIMD); O_sb = O_T * r (DVE) -> DMA out as O_T [D, N]; host transposes.
"""

import sys

sys.path.insert(0, "/opt/trn_rl_repo")

import numpy as np
import ml_dtypes

import concourse.bass as bass
import concourse.tile as tile
from concourse import bacc, mybir
from concourse.bass_utils import run_bass_kernel_spmd

B, H, N, D = 4, 16, 2048, 128
NCORES = 8
HPC = (B * H) // NCORES  # heads per core = 8
P = 128                  # partitions
NK = N // P              # key chunks per head = 16
QH = 2                   # q halves (1024 each) to fit PSUM
QHW = N // QH            # 1024
F32 = mybir.dt.float32
BF16 = mybir.dt.bfloat16
FP16 = mybir.dt.float16


def build_nc():
    nc = bacc.Bacc(None, target_bir_lowering=False)

    q1_d = nc.dram_tensor("q1", [HPC, D, N], FP16, kind="ExternalInput")
    q2_d = nc.dram_tensor("q2", [HPC, D, N], FP16, kind="ExternalInput")
    k1_d = nc.dram_tensor("k1", [HPC, D, N], FP16, kind="ExternalInput")
    k2_d = nc.dram_tensor("k2", [HPC, D, N], FP16, kind="ExternalInput")
    v_d = nc.dram_tensor("v", [HPC, N, D], FP16, kind="ExternalInput")
    ot_d = nc.dram_tensor("ot", [HPC, D, N], F32, kind="ExternalOutput")

    with tile.TileContext(nc) as tc:
        with (
            tc.tile_pool(name="const", bufs=1) as const_pool,
            tc.tile_pool(name="io", bufs=2) as io_pool,
            tc.tile_pool(name="e", bufs=18) as e_pool,
            tc.tile_pool(name="osb", bufs=2) as o_pool,
            tc.tile_pool(name="small", bufs=2) as small_pool,
            tc.tile_pool(name="ps_s", bufs=2, space="PSUM") as ps_s_pool,
            tc.tile_pool(name="ps_o", bufs=1, space="PSUM") as ps_o_pool,
            tc.tile_pool(name="ps_l", bufs=1, space="PSUM") as ps_l_pool,
        ):
            ones_col = const_pool.tile([P, 1], FP16)  # sum weights
            nc.vector.memset(ones_col[:], 1.0)
            mask4 = const_pool.tile([P, 1], BF16)     # combine weights
            nc.vector.memset(mask4[:], 0.0)
            for g in range(4):
                nc.vector.memset(mask4[32 * g: 32 * g + 1, :], 1.0)

            SUMB = 16  # chunks per column-tiled row-sum batch

            def load_head(h):
                q1t = io_pool.tile([P, N], FP16, tag="q1")
                nc.sync.dma_start(out=q1t[:], in_=q1_d[h])
                q2t = io_pool.tile([P, N], FP16, tag="q2")
                nc.sync.dma_start(out=q2t[:], in_=q2_d[h])
                k1t = io_pool.tile([P, N], FP16, tag="k1")
                nc.sync.dma_start(out=k1t[:], in_=k1_d[h])
                k2t = io_pool.tile([P, N], FP16, tag="k2")
                nc.sync.dma_start(out=k2t[:], in_=k2_d[h])
                # vt[p, c, d] = V[h, c*128 + p, d]
                vt3 = io_pool.tile([P, NK, P], FP16, tag="vt")
                nc.sync.dma_start(
                    out=vt3[:], in_=v_d[h].rearrange("(c p) d -> p c d", p=P)
                )
                return q1t, q2t, k1t, k2t, vt3.rearrange("p c d -> p (c d)")

            def make_tail(ps_o, ps_l, h, q0):
                def tail():
                    # combine 4 partial rows: l = mask4.T @ (l4_hi + l4_lo) --
                    # bf16 hi/lo split keeps the combine matmuls bf16-fast
                    # while preserving ~17 bits of l. Then r = 1/l (DVE
                    # approx, ~2 ULP), broadcast across partitions (GPSIMD),
                    # O = O_T * r (DVE), store.
                    l4_hi = small_pool.tile([P, QHW], BF16, tag="l4h")
                    nc.vector.tensor_copy(l4_hi[:], ps_l[:])
                    l4_lo = small_pool.tile([P, QHW], BF16, tag="l4l")
                    nc.vector.scalar_tensor_tensor(
                        out=l4_lo[:],
                        in0=ps_l[:],
                        scalar=1.0,
                        in1=l4_hi[:],
                        op0=mybir.AluOpType.mult,
                        op1=mybir.AluOpType.subtract,
                    )
                    ps_lc = ps_s_pool.tile([P, QHW], F32, tag="s")
                    for pi, part in enumerate((l4_hi, l4_lo)):
                        for j in range(2):
                            sl = slice(j * 512, (j + 1) * 512)
                            nc.tensor.matmul(
                                ps_lc[0:1, sl], mask4[:], part[:, sl],
                                start=(pi == 0), stop=(pi == 1),
                            )
                    r_sb = small_pool.tile([1, QHW], F32, tag="r")
                    scratch = small_pool.tile([1, QHW], F32, tag="rs")
                    nc.vector.reciprocal_approx_accurate(
                        r_sb[:], ps_lc[0:1, :], scratch[:]
                    )
                    r_bc = small_pool.tile([P, QHW], F32, tag="rbc")
                    nc.gpsimd.partition_broadcast(r_bc[:], r_sb[:])
                    o_sb = o_pool.tile([P, QHW], F32, tag="osb")
                    nc.vector.tensor_mul(o_sb[:], ps_o[:], r_bc[:])
                    nc.sync.dma_start(out=ot_d[h][:, q0: q0 + QHW], in_=o_sb[:])
                return tail

            pending_tail = None
            tiles = None
            for h in range(HPC):
                for qh in range(QH):
                    if qh == 0:
                        tiles = load_head(h)
                    q1t, q2t, k1t, k2t, vt = tiles
                    q0 = qh * QHW
                    ps_o = ps_o_pool.tile([P, QHW], F32, tag="o")
                    ps_l = ps_l_pool.tile([P, QHW], F32, tag="l")
                    e_tiles = []

                    def pv(c):
                        cs2 = slice(c * P, (c + 1) * P)
                        for j in range(2):
                            sl = slice(j * 512, (j + 1) * 512)
                            nc.tensor.matmul(
                                ps_o[:, sl],
                                vt[:, cs2],
                                e_tiles[c][:, sl],
                                start=(c == 0),
                                stop=(c == NK - 1),
                            )

                    for c in range(NK):
                        cs = slice(c * P, (c + 1) * P)
                        ps_s = ps_s_pool.tile([P, QHW], F32, tag="s")
                        # 3-term hi/lo split of S; grouped by lhsT for reuse
                        terms = [
                            (k1t[:, cs], q1t, False),
                            (k1t[:, cs], q2t, False),
                            (k2t[:, cs], q1t, True),
                        ]
                        for t, (kc, qt, last) in enumerate(terms):
                            for j in range(2):
                                sl = slice(j * 512, (j + 1) * 512)
                                nc.tensor.matmul(
                                    ps_s[:, sl],
                                    kc,
                                    qt[:, q0 + j * 512: q0 + (j + 1) * 512],
                                    start=(t == 0),
                                    stop=last,
                                )
                        e = e_pool.tile([P, QHW], BF16, tag="e")
                        nc.scalar.activation(
                            e[:], ps_s[:], mybir.ActivationFunctionType.Exp
                        )
                        e_tiles.append(e)
                        # PV for the previous chunk: its exp finished while
                        # this chunk's S-matmuls streamed, so the PE never
                        # waits on the ACT engine.
                        if c > 0:
                            pv(c - 1)
                        # previous round's normalization tail, deferred here so
                        # its DVE/GPSIMD latency hides behind this round's
                        # S-matmul stream instead of stalling the PE.
                        if c == 1 and pending_tail is not None:
                            pending_tail()
                            pending_tail = None
                    pv(NK - 1)
                    # Column-tiled row sums, batched: the four M=1 matmul
                    # groups (PE column groups / output partitions
                    # 0/32/64/96) are issued in waves of 4 so distinct
                    # groups stream concurrently through the array.
                    for j in range(2):
                        sl = slice(j * 512, (j + 1) * 512)
                        for rep in range(NK // 4):
                            for g in range(4):
                                nc.tensor.matmul(
                                    ps_l[32 * g: 32 * g + 1, sl],
                                    ones_col[:],
                                    e_tiles[rep * 4 + g][:, sl],
                                    start=(rep == 0),
                                    stop=(rep == NK // 4 - 1),
                                    tile_position=(0, 32 * g),
                                )
                    pending_tail = make_tail(ps_o, ps_l, h, q0)
            pending_tail()
    nc.finalize()
    return nc


def _split_fp16_t(x):
    """[heads, N, D] fp32 -> transposed [heads, D, N] fp16 hi and lo parts."""
    xt = np.ascontiguousarray(x.transpose(0, 2, 1))
    hi = xt.astype(np.float16)
    lo = (xt - hi.astype(np.float32)).astype(np.float16)
    return hi, lo


def _prepare_in_maps(Q, K, V):
    Qf = np.asarray(Q, dtype=np.float32).reshape(B * H, N, D)
    Kf = np.asarray(K, dtype=np.float32).reshape(B * H, N, D)
    Vf = np.asarray(V, dtype=np.float32).reshape(B * H, N, D).astype(np.float16)
    q1, q2 = _split_fp16_t(Qf)
    k1, k2 = _split_fp16_t(Kf)
    in_maps = []
    for i in range(NCORES):
        s = slice(i * HPC, (i + 1) * HPC)
        in_maps.append(
            {"q1": q1[s], "q2": q2[s], "k1": k1[s], "k2": k2[s], "v": Vf[s]}
        )
    return in_maps


def run(Q, K, V, trace=False, **kwargs):
    nc = build_nc()
    in_maps = _prepare_in_maps(Q, K, V)
    res = run_bass_kernel_spmd(nc, in_maps, list(range(NCORES)), trace=trace, **kwargs)
    OT = np.concatenate([res.results[i]["ot"] for i in range(NCORES)], axis=0)
    out = OT.transpose(0, 2, 1).reshape(B, H, N, D)
    return np.ascontiguousarray(out), res


def kernel(Q, K, V):
    out, _ = run(Q, K, V, trace=False)
    return out


# revision 26
# speedup vs baseline: 1.4566x; 1.1376x over previous
"""Trainium2 Bass kernel: batched attention  out = softmax(Q K^T) V  (no 1/sqrt(d) scale).

Shapes (hardcoded): Q, K, V: [4, 16, 2048, 128] fp32 -> out [4, 16, 2048, 128] fp32.

Sharding: B*H = 64 heads, data-parallel across 8 NeuronCores (8 heads per core).

Per-head device algorithm (transpose-free matmul layout, 16-bit PE with hi/lo
split for the accuracy-critical S = Q K^T):
  Host pre-transposes Q, K to [D, N] per head and splits each into fp16
  hi + lo parts (q = q1 + q2 exactly to ~22 mantissa bits). V is sent fp16
  (values O(1): fp16 range fine, 2^-11 rounding).
  For each 128-wide key chunk c (dropped q2*k2 term ~2^-22):
      S_T[c]  = k1c.T @ q1 + k1c.T @ q2 + k2c.T @ q1   -> PSUM [128k, q] fp32
      E[c]    = exp(S_T[c])  (ACT; bf16 out -- bf16 covers exp range e^+-70;
                no max-subtract needed)
      O_T    += vc.T @ E[c]                     (PSUM accumulate, fp32)
      l4[g]  += ones.T @ E[c],  g = c mod 4     (4-way column-tiled row sums:
                the 4 M=1 matmuls stream concurrently in distinct PE column
                groups, output partitions 0/32/64/96)
  l = mask4.T @ l4 (fp32 matmul combining the 4 partial rows)
  r = approx-reciprocal(l) (DVE, ~2 ULP); broadcast across partitions
  (GPSIMD); O_sb = O_T * r (DVE) -> DMA out as O_T [D, N]; host transposes.
"""

import sys

sys.path.insert(0, "/opt/trn_rl_repo")

import numpy as np
import ml_dtypes

import concourse.bass as bass
import concourse.tile as tile
from concourse import bacc, mybir
from concourse.bass_utils import run_bass_kernel_spmd

B, H, N, D = 4, 16, 2048, 128
NCORES = 8
HPC = (B * H) // NCORES  # heads per core = 8
P = 128                  # partitions
NK = N // P              # key chunks per head = 16
QH = 2                   # q halves (1024 each) to fit PSUM
QHW = N // QH            # 1024
F32 = mybir.dt.float32
BF16 = mybir.dt.bfloat16
FP16 = mybir.dt.float16
FP8 = mybir.dt.float8e5


def build_nc():
    nc = bacc.Bacc(None, target_bir_lowering=False)

    q1_d = nc.dram_tensor("q1", [HPC, D, N], FP16, kind="ExternalInput")
    qx_d = nc.dram_tensor("qx", [HPC, D, 2, N], FP8, kind="ExternalInput")
    k1_d = nc.dram_tensor("k1", [HPC, D, N], FP16, kind="ExternalInput")
    kx_d = nc.dram_tensor("kx", [HPC, D, 2, N], FP8, kind="ExternalInput")
    v_d = nc.dram_tensor("v", [HPC, N, D], FP16, kind="ExternalInput")
    ot_d = nc.dram_tensor("ot", [HPC, D, N], F32, kind="ExternalOutput")

    with tile.TileContext(nc) as tc:
        with (
            tc.tile_pool(name="const", bufs=1) as const_pool,
            tc.tile_pool(name="io", bufs=2) as io_pool,
            tc.tile_pool(name="e", bufs=18) as e_pool,
            tc.tile_pool(name="osb", bufs=2) as o_pool,
            tc.tile_pool(name="small", bufs=2) as small_pool,
            tc.tile_pool(name="ps_s", bufs=2, space="PSUM") as ps_s_pool,
            tc.tile_pool(name="ps_o", bufs=1, space="PSUM") as ps_o_pool,
            tc.tile_pool(name="ps_l", bufs=1, space="PSUM") as ps_l_pool,
        ):
            ones_col = const_pool.tile([P, 1], FP16)  # sum weights
            nc.vector.memset(ones_col[:], 1.0)
            mask4 = const_pool.tile([P, 1], BF16)     # combine weights
            nc.vector.memset(mask4[:], 0.0)
            for g in range(4):
                nc.vector.memset(mask4[32 * g: 32 * g + 1, :], 1.0)

            SUMB = 16  # chunks per column-tiled row-sum batch

            def load_head(h):
                q1t = io_pool.tile([P, N], FP16, tag="q1")
                nc.sync.dma_start(out=q1t[:], in_=q1_d[h])
                qxt = io_pool.tile([P, 2, N], FP8, tag="qx")
                nc.sync.dma_start(out=qxt[:], in_=qx_d[h])
                k1t = io_pool.tile([P, N], FP16, tag="k1")
                nc.sync.dma_start(out=k1t[:], in_=k1_d[h])
                kxt = io_pool.tile([P, 2, N], FP8, tag="kx")
                nc.sync.dma_start(out=kxt[:], in_=kx_d[h])
                # vt[p, c, d] = V[h, c*128 + p, d]
                vt3 = io_pool.tile([P, NK, P], FP16, tag="vt")
                nc.sync.dma_start(
                    out=vt3[:], in_=v_d[h].rearrange("(c p) d -> p c d", p=P)
                )
                return q1t, qxt, k1t, kxt, vt3.rearrange("p c d -> p (c d)")

            def make_tail(ps_o, ps_l, h, q0):
                def tail():
                    # combine 4 partial rows: l = mask4.T @ (l4_hi + l4_lo) --
                    # bf16 hi/lo split keeps the combine matmuls bf16-fast
                    # while preserving ~17 bits of l. Then r = 1/l (DVE
                    # approx, ~2 ULP), broadcast across partitions (GPSIMD),
                    # O = O_T * r (DVE), store.
                    l4_hi = small_pool.tile([P, QHW], BF16, tag="l4h")
                    nc.vector.tensor_copy(l4_hi[:], ps_l[:])
                    l4_lo = small_pool.tile([P, QHW], BF16, tag="l4l")
                    nc.vector.scalar_tensor_tensor(
                        out=l4_lo[:],
                        in0=ps_l[:],
                        scalar=1.0,
                        in1=l4_hi[:],
                        op0=mybir.AluOpType.mult,
                        op1=mybir.AluOpType.subtract,
                    )
                    ps_lc = ps_s_pool.tile([P, QHW], F32, tag="s")
                    for pi, part in enumerate((l4_hi, l4_lo)):
                        for j in range(2):
                            sl = slice(j * 512, (j + 1) * 512)
                            nc.tensor.matmul(
                                ps_lc[0:1, sl], mask4[:], part[:, sl],
                                start=(pi == 0), stop=(pi == 1),
                            )
                    r_sb = small_pool.tile([1, QHW], F32, tag="r")
                    scratch = small_pool.tile([1, QHW], F32, tag="rs")
                    nc.vector.reciprocal_approx_accurate(
                        r_sb[:], ps_lc[0:1, :], scratch[:]
                    )
                    r_bc = small_pool.tile([P, QHW], F32, tag="rbc")
                    nc.gpsimd.partition_broadcast(r_bc[:], r_sb[:])
                    o_sb = o_pool.tile([P, QHW], F32, tag="osb")
                    nc.vector.tensor_mul(o_sb[:], ps_o[:], r_bc[:])
                    nc.sync.dma_start(out=ot_d[h][:, q0: q0 + QHW], in_=o_sb[:])
                return tail

            pending_tail = None
            tiles = None
            for h in range(HPC):
                for qh in range(QH):
                    if qh == 0:
                        tiles = load_head(h)
                    q1t, qxt, k1t, kxt, vt = tiles
                    q0 = qh * QHW
                    ps_o = ps_o_pool.tile([P, QHW], F32, tag="o")
                    ps_l = ps_l_pool.tile([P, QHW], F32, tag="l")
                    e_tiles = []

                    def pv(c):
                        cs2 = slice(c * P, (c + 1) * P)
                        for j in range(2):
                            sl = slice(j * 512, (j + 1) * 512)
                            nc.tensor.matmul(
                                ps_o[:, sl],
                                vt[:, cs2],
                                e_tiles[c][:, sl],
                                start=(c == 0),
                                stop=(c == NK - 1),
                            )

                    for c in range(NK):
                        cs = slice(c * P, (c + 1) * P)
                        ps_s = ps_s_pool.tile([P, QHW], F32, tag="s")
                        # 2-stream hi/lo split of S: fp16 hi term k1.T @ q1,
                        # plus BOTH fp8-e5m2 cross terms (k1.T @ q2 + k2.T @
                        # q1) in one DoubleRow matmul -- operand pairs (k1,
                        # k2) x (q2, q1) packed along the interleave axis
                        # contract K=256 in a single 512-cycle stream. The
                        # cross terms are ~2^-11-scale corrections, so fp8
                        # rounding on them is second-order.
                        for j in range(2):
                            sl = slice(j * 512, (j + 1) * 512)
                            nc.tensor.matmul(
                                ps_s[:, sl],
                                k1t[:, cs],
                                q1t[:, q0 + j * 512: q0 + (j + 1) * 512],
                                start=True,
                                stop=False,
                            )
                        for j in range(2):
                            sl = slice(j * 512, (j + 1) * 512)
                            nc.tensor.matmul(
                                ps_s[:, sl],
                                kxt[:, :, cs],
                                qxt[:, :, q0 + j * 512: q0 + (j + 1) * 512],
                                start=False,
                                stop=True,
                                perf_mode=mybir.MatmulPerfMode.DoubleRow,
                            )
                        e = e_pool.tile([P, QHW], BF16, tag="e")
                        nc.scalar.activation(
                            e[:], ps_s[:], mybir.ActivationFunctionType.Exp
                        )
                        e_tiles.append(e)
                        # PV for the previous chunk: its exp finished while
                        # this chunk's S-matmuls streamed, so the PE never
                        # waits on the ACT engine.
                        if c > 0:
                            pv(c - 1)
                        # previous round's normalization tail, deferred here so
                        # its DVE/GPSIMD latency hides behind this round's
                        # S-matmul stream instead of stalling the PE.
                        if c == 2 and pending_tail is not None:
                            pending_tail()
                            pending_tail = None
                    pv(NK - 1)
                    # Column-tiled row sums, batched: the four M=1 matmul
                    # groups (PE column groups / output partitions
                    # 0/32/64/96) are issued in waves of 4 so distinct
                    # groups stream concurrently through the array.
                    for j in range(2):
                        sl = slice(j * 512, (j + 1) * 512)
                        for rep in range(NK // 4):
                            for g in range(4):
                                nc.tensor.matmul(
                                    ps_l[32 * g: 32 * g + 1, sl],
                                    ones_col[:],
                                    e_tiles[rep * 4 + g][:, sl],
                                    start=(rep == 0),
                                    stop=(rep == NK // 4 - 1),
                                    tile_position=(0, 32 * g),
                                )
                    pending_tail = make_tail(ps_o, ps_l, h, q0)
            pending_tail()
    nc.finalize()
    return nc


E5M2 = ml_dtypes.float8_e5m2


def _split_fp16_t(x):
    """[heads, N, D] fp32 -> transposed [heads, D, N] fp16 hi + fp32 lo."""
    xt = np.ascontiguousarray(x.transpose(0, 2, 1))
    hi = xt.astype(np.float16)
    lo = xt - hi.astype(np.float32)
    return hi, lo


def _prepare_in_maps(Q, K, V):
    Qf = np.asarray(Q, dtype=np.float32).reshape(B * H, N, D)
    Kf = np.asarray(K, dtype=np.float32).reshape(B * H, N, D)
    Vf = np.asarray(V, dtype=np.float32).reshape(B * H, N, D).astype(np.float16)
    q1, q2 = _split_fp16_t(Qf)
    k1, k2 = _split_fp16_t(Kf)
    # fp8 cross-term operands, paired along the DoubleRow interleave axis:
    # weights (k1, k2) x moving (q2, q1) -> k1.T@q2 + k2.T@q1
    qx = np.stack([q2.astype(E5M2), np.asarray(q1).astype(E5M2)], axis=2)
    kx = np.stack([np.asarray(k1).astype(E5M2), k2.astype(E5M2)], axis=2)
    in_maps = []
    for i in range(NCORES):
        s = slice(i * HPC, (i + 1) * HPC)
        in_maps.append(
            {"q1": q1[s], "qx": qx[s], "k1": k1[s], "kx": kx[s], "v": Vf[s]}
        )
    return in_maps


def run(Q, K, V, trace=False, **kwargs):
    nc = build_nc()
    in_maps = _prepare_in_maps(Q, K, V)
    res = run_bass_kernel_spmd(nc, in_maps, list(range(NCORES)), trace=trace, **kwargs)
    OT = np.concatenate([res.results[i]["ot"] for i in range(NCORES)], axis=0)
    out = OT.transpose(0, 2, 1).reshape(B, H, N, D)
    return np.ascontiguousarray(out), res


def kernel(Q, K, V):
    out, _ = run(Q, K, V, trace=False)
    return out
